# revision 13
# baseline (speedup 1.0000x reference)
"""Trainium2 Bass kernel for nn_MemoryCell (scatter_memory).

Full-input contract: kernel(**inputs) takes the complete (unsharded) numpy
inputs and returns the full [NB*B, H] output.

Math (B == H == 1024, NB == 5, T == 128):
    enc  = features[:, 0, :]                         # [B, H] - only slice used
    h    = states.reshape(NB, H)
    gate = sigmoid(enc @ (h + keys).T)               # [B, NB]
    pre  = (h @ Uw.T + keys @ Vw.T)[:, None, :] + (enc @ Ww.T)[None, :, :]
    cand = where(pre >= 0, pre, prelu_a * pre)
    new[i, b, j] = h[i, j] + gate[j, i] * cand[i, b, j]   # B==H broadcast quirk
    out  = sign(new) with exact zeros -> +1, reshaped [NB*B, H]

Because the output is pure signs, the elementwise tail collapses to a
per-(i, j) THRESHOLD on ew = enc @ Ww.T:
    out[i, b, j] = +1  iff  ew[b, j] >= THR[i, j]
with THR = t_cand - huv, t_cand = (q >= 0 ? q : q / a_j), q = -h/gate
(valid for prelu_a > 0; PReLU is monotone there).  gate/huv/THR involve only
O(H*NB) work on tiny tensors and sit on the host (float64, exact), applied
during the gather/unshard step along with the sign expansion.

The device work per core is ONE [512, 1024] x [1024, 256] GEMM in plain fp16
(both operands round-to-nearest fp16: the PE upconverts fp16 to FP22
losslessly, so HW matches the host simulation; together with the fp16
round-trip of ew itself this measures 132 sign flips of 5.24M, rel err
0.010 vs the 0.02 gate) and ships ew back as fp16 (0.26 MB/core).

Sharding: 2 b-halves x 4 j-quarters = 8 cores.  Per core DMA: Ww quarter
(0.5 MB fp16) + enc half (1 MB fp16) in, ew (0.26 MB fp16) out.  Inputs
stream k-chunk-paced on BOTH HWDGE rings (sync + scalar) so the matmul
series chases the arrivals; a short identity warm-up keeps the PE HAM
activity window busy so the series runs at the warm clock.
"""

import os
import numpy as np

H = 1024
B = 1024
NB = 5
NCORES = 8
NJ = 4              # j-quarters of 256 columns
NBH = 2             # b-halves of 512 rows
BS = 256            # b sub-chunk (PSUM tile width)

_NC_CACHE = {}


def _build_nc():
    from concourse import bacc, mybir
    import concourse.tile as tile

    f32 = mybir.dt.float32
    f16 = mybir.dt.float16
    AF = mybir.ActivationFunctionType

    nc = bacc.Bacc("TRN2", debug=False, num_devices=NCORES)

    # The DMA fabric maps partition p -> SDMA engine 64 + p//8, and engine
    # 79 is a chronic straggler (its 2:1 mux partner serves another stream),
    # so every 128-partition DMA's completion semaphore trails its bytes by
    # ~2 us.  All INPUT transfers therefore use partitions 0..119 only:
    # the contraction k is chunked as 8 x 120 + 1 x 64 (ragged, no padding).
    # enc pieces per sub: chunks (0-2), (3-5), (6-7), (8).
    encd0 = nc.dram_tensor("encd0", [2, 120, 3, BS], f16, kind="ExternalInput").ap()
    encd1 = nc.dram_tensor("encd1", [2, 120, 3, BS], f16, kind="ExternalInput").ap()
    encd2 = nc.dram_tensor("encd2", [2, 120, 2, BS], f16, kind="ExternalInput").ap()
    encd3 = nc.dram_tensor("encd3", [2, 64, 1, BS], f16, kind="ExternalInput").ap()
    # weights [p, chunk, jt, j]: chunks (0-4) / (5-7) / (8)
    wda = nc.dram_tensor("wda", [120, 5, 2, 128], f16, kind="ExternalInput").ap()
    wdb1 = nc.dram_tensor("wdb1", [120, 3, 2, 128], f16, kind="ExternalInput").ap()
    wdb2 = nc.dram_tensor("wdb2", [64, 1, 2, 128], f16, kind="ExternalInput").ap()
    outd = nc.dram_tensor("out", [2, 128, 2, BS], f16, kind="ExternalOutput").ap()

    with tile.TileContext(nc) as tc:
        with (
            tc.tile_pool(name="res", bufs=1) as res,
            tc.tile_pool(name="work", bufs=1) as work,
            tc.tile_pool(name="pp", bufs=1, space="PSUM") as pp,
        ):
            # ---- input DMAs on both HWDGE rings, weights first ----
            wA = res.tile([120, 5, 2, 128], f16, name="wA")
            nc.sync.dma_start(wA, wda)
            wB1 = res.tile([120, 3, 2, 128], f16, name="wB1")
            nc.scalar.dma_start(wB1, wdb1)
            wB2 = res.tile([64, 1, 2, 128], f16, name="wB2")
            nc.scalar.dma_start(wB2, wdb2)
            eshapes = [(120, 3), (120, 3), (120, 2), (64, 1)]
            edrams = [encd0, encd1, encd2, encd3]
            e_t = [[], []]
            for s in range(2):
                for pi, (pd, nk) in enumerate(eshapes):
                    e = res.tile([pd, nk, BS], f16, name=f"e{s}{pi}")
                    (nc.sync if s == 0 else nc.scalar).dma_start(
                        e, edrams[pi][s])
                    e_t[s].append(e)

            def w_sl(k, t):
                if k < 5:
                    return wA[:, k, t, :]
                if k < 8:
                    return wB1[:, k - 5, t, :]
                return wB2[:, 0, t, :]

            def e_sl(s, k):
                pi = 3 if k == 8 else k // 3
                return e_t[s][pi][:, k - (0, 3, 6, 8)[pi], :]

            # PE warm-up while the stream lands: identity built on-chip
            # (a DMA'd identity has 256 B partition lines - RMW-slow - and
            # clogs the ring head); the run keeps the HAM activity window
            # busy so the real series starts at the warm clock
            from concourse.masks import make_identity
            id_sb = res.tile([128, 128], f16, name="id_sb")
            make_identity(nc, id_sb)
            psum_warm = pp.tile([128, 128], f32, name="psum_warm")
            for _ in range(20):
                nc.tensor.matmul(psum_warm, lhsT=id_sb, rhs=id_sb,
                                 start=True, stop=True)

            ps = [[pp.tile([128, BS], f32, name=f"ps{s}{t}") for t in range(2)]
                  for s in range(2)]
            ew_sb = [work.tile([128, 2, BS], f16, name=f"ew{s}")
                     for s in range(2)]

            # k-major, subs interleaved: the series chases piece arrivals on
            # both rings, so after the last piece's semaphore only the final
            # chunk's matmuls remain
            for k in range(9):
                for s in range(2):
                    for t in range(2):
                        nc.tensor.matmul(
                            ps[s][t], lhsT=w_sl(k, t), rhs=e_sl(s, k),
                            start=(k == 0), stop=(k == 8))
            for s in range(2):
                nc.scalar.activation(ew_sb[s][:, 0, :], ps[s][0], AF.Copy)
                nc.vector.tensor_copy(out=ew_sb[s][:, 1, :], in_=ps[s][1])
                (nc.sync if s == 0 else nc.scalar).dma_start(outd[s], ew_sb[s])

    nc.compile()
    return nc


def _get_nc():
    nc = _NC_CACHE.get("nc")
    if nc is None:
        nc = _build_nc()
        _NC_CACHE["nc"] = nc
    return nc


# contraction chunk boundaries: 8 x 120 + 1 x 64 (partitions 0..119 only,
# keeping SDMA engine 79 out of the input path)
_CO = [0, 120, 240, 360, 480, 600, 720, 840, 960, 1024]
_PIECES = [(0, 3), (3, 3), (6, 2), (8, 1)]   # (first chunk, n chunks)


def _chunk_stack(mat, c0, nc_, width):
    # mat [1024 k, width] -> [pd, nc_, width] for chunks c0..c0+nc_-1
    pd = _CO[c0 + 1] - _CO[c0]
    out = np.empty((pd, nc_, width), dtype=np.float16)
    for i in range(nc_):
        c = c0 + i
        out[:_CO[c + 1] - _CO[c], i] = mat[_CO[c]:_CO[c + 1]]
    return out


def _pack_enc(enc_half):
    # [512 b, 1024 k] f32 -> list of 4 piece arrays [2, pd, nk, 256] f16
    e = np.ascontiguousarray(enc_half.T).astype(np.float16)   # [k, b]
    pieces = []
    for c0, nk in _PIECES:
        arr = np.stack([_chunk_stack(e[:, s * BS:(s + 1) * BS], c0, nk, BS)
                        for s in range(2)])
        pieces.append(np.ascontiguousarray(arr))
    return pieces


def _pack_w(Wq):
    # [256 j, 1024 k] f32 -> (wda [120,5,2,128], wdb1 [120,3,2,128],
    #                         wdb2 [64,1,2,128]) f16
    w = np.ascontiguousarray(Wq.T).astype(np.float16)         # [k, j]
    w = w.reshape(1024, 2, 128).reshape(1024, -1)             # [k, jt*j]
    wa = _chunk_stack(w, 0, 5, 256).reshape(120, 5, 2, 128)
    wb1 = _chunk_stack(w, 5, 3, 256).reshape(120, 3, 2, 128)
    wb2 = _chunk_stack(w, 8, 1, 256).reshape(64, 1, 2, 128)
    return (np.ascontiguousarray(wa), np.ascontiguousarray(wb1),
            np.ascontiguousarray(wb2))


def _host_fallback(enc, h, keys, Uw, Vw, Ww, prelu_a):
    # exact reference math (only used if prelu_a has non-positive entries,
    # where the threshold fold is invalid; never hit for the spec'd inputs)
    gate = 1.0 / (1.0 + np.exp(-(enc @ (h + keys).T)))
    pre = (h @ Uw.T + keys @ Vw.T)[:, None, :] + (enc @ Ww.T)[None, :, :]
    cand = np.where(pre >= 0, pre, prelu_a * pre)
    new = h[:, None, :] + gate.T[:, None, :] * cand
    new = np.where(new == 0, 0.1, new)
    return np.where(new >= 0, np.float32(1.0), np.float32(-1.0)).reshape(
        NB * B, H).astype(np.float32)


def kernel(features, states, Uw, Vw, Ww, keys, prelu_a):
    from concourse import bass_utils

    features = np.asarray(features)
    states = np.asarray(states, dtype=np.float32)
    Uw = np.asarray(Uw, dtype=np.float32)
    Vw = np.asarray(Vw, dtype=np.float32)
    Ww = np.asarray(Ww, dtype=np.float32)
    keys = np.asarray(keys, dtype=np.float32)
    prelu_a = np.asarray(prelu_a, dtype=np.float32)

    enc = np.ascontiguousarray(features[:, 0, :], dtype=np.float32)  # [B, H]
    h = states.reshape(NB, H)

    if np.any(prelu_a <= 0):
        return _host_fallback(enc.astype(np.float64), h.astype(np.float64),
                              keys.astype(np.float64), Uw.astype(np.float64),
                              Vw.astype(np.float64), Ww.astype(np.float64),
                              prelu_a.astype(np.float64))

    # ---- tiny tensors -> per-(i, j) thresholds, in float64 ----
    enc64 = enc.astype(np.float64)
    h64 = h.astype(np.float64)
    k64 = keys.astype(np.float64)
    gateT = 1.0 / (1.0 + np.exp(-(enc64 @ (h64 + k64).T))).T      # [i, j]
    huv = h64 @ Uw.astype(np.float64).T + k64 @ Vw.astype(np.float64).T
    q = -h64 / gateT
    a = prelu_a.astype(np.float64)[None, :]
    t_cand = np.where(q >= 0, q, q / a)
    THR = np.clip(t_cand - huv, -1e30, 1e30).astype(np.float32)   # [i, j]

    nc = _get_nc()

    enc_packs = [_pack_enc(enc[bh * 512:(bh + 1) * 512]) for bh in range(NBH)]
    w_packs = [_pack_w(Ww[jq * 256:(jq + 1) * 256]) for jq in range(NJ)]
    in_maps = []
    for c in range(NCORES):
        jq, bh = divmod(c, 2)
        ep = enc_packs[bh]
        wa, wb1, wb2 = w_packs[jq]
        in_maps.append({
            "encd0": ep[0], "encd1": ep[1], "encd2": ep[2], "encd3": ep[3],
            "wda": wa, "wdb1": wb1, "wdb2": wb2,
        })

    trace = bool(int(os.environ.get("KERNEL_TRACE", "0")))
    res = bass_utils.run_bass_kernel_spmd(
        nc, in_maps, core_ids=list(range(NCORES)), trace=trace)
    kernel.last_result = res

    # gather ew [b, j] from the cores, then apply the thresholds
    ew = np.empty((B, H), dtype=np.float32)
    ev = ew.reshape(NBH, 2, BS, NJ, 2, 128)    # [bh, s, col, jq, t, p]
    for c in range(NCORES):
        jq, bh = divmod(c, 2)
        o = res.results[c]["out"]              # [s, p, t, col] f16
        ev[bh, :, :, jq, :, :] = o.transpose(0, 3, 2, 1)  # [s, col, t, p]
    one = np.float32(1.0)
    neg = np.float32(-1.0)
    full = np.where(ew[None, :, :] >= THR[:, None, :], one, neg)
    return np.ascontiguousarray(full.reshape(NB * B, H), dtype=np.float32)


# revision 16
# speedup vs baseline: 1.0469x; 1.0469x over previous
"""Trainium2 Bass kernel for nn_MemoryCell (scatter_memory).

Full-input contract: kernel(**inputs) takes the complete (unsharded) numpy
inputs and returns the full [NB*B, H] output.

Math (B == H == 1024, NB == 5, T == 128):
    enc  = features[:, 0, :]                         # [B, H] - only slice used
    h    = states.reshape(NB, H)
    gate = sigmoid(enc @ (h + keys).T)               # [B, NB]
    pre  = (h @ Uw.T + keys @ Vw.T)[:, None, :] + (enc @ Ww.T)[None, :, :]
    cand = where(pre >= 0, pre, prelu_a * pre)
    new[i, b, j] = h[i, j] + gate[j, i] * cand[i, b, j]   # B==H broadcast quirk
    out  = sign(new) with exact zeros -> +1, reshaped [NB*B, H]

Because the output is pure signs, the elementwise tail collapses to a
per-(i, j) THRESHOLD on ew = enc @ Ww.T:
    out[i, b, j] = +1  iff  ew[b, j] >= THR[i, j]
with THR = t_cand - huv, t_cand = (q >= 0 ? q : q / a_j), q = -h/gate
(valid for prelu_a > 0; PReLU is monotone there).  gate/huv/THR involve only
O(H*NB) work on tiny tensors and sit on the host (float64, exact), applied
during the gather/unshard step along with the sign expansion.

The device work per core is ONE [512, 1024] x [1024, 256] GEMM in plain fp16
(both operands round-to-nearest fp16: the PE upconverts fp16 to FP22
losslessly, so HW matches the host simulation; together with the fp16
round-trip of ew itself this measures 132 sign flips of 5.24M, rel err
0.010 vs the 0.02 gate) and ships ew back as fp16 (0.26 MB/core).

Sharding: 2 b-halves x 4 j-quarters = 8 cores.  Per core DMA: Ww quarter
(0.5 MB fp16) + enc half (1 MB fp16) in, ew (0.26 MB fp16) out.  Inputs
stream k-chunk-paced on BOTH HWDGE rings (sync + scalar) so the matmul
series chases the arrivals; a short identity warm-up keeps the PE HAM
activity window busy so the series runs at the warm clock.
"""

import os
import numpy as np

H = 1024
B = 1024
NB = 5
NCORES = 8
NJ = 4              # j-quarters of 256 columns
NBH = 2             # b-halves of 512 rows
BS = 256            # b sub-chunk (PSUM tile width)

_NC_CACHE = {}


def _build_nc():
    from concourse import bacc, mybir
    import concourse.tile as tile

    f32 = mybir.dt.float32
    f16 = mybir.dt.float16
    AF = mybir.ActivationFunctionType

    nc = bacc.Bacc("TRN2", debug=False, num_devices=NCORES)

    # g = sub*4 + kq; [p, kl, col] with k = (kq*2 + kl)*128 + p
    encd = nc.dram_tensor("encd", [8, 128, 2, BS], f16, kind="ExternalInput").ap()
    # [khalf, p, kl, jt, j]
    wd = nc.dram_tensor("wd", [2, 128, 4, 2, 128], f16, kind="ExternalInput").ap()
    outd = nc.dram_tensor("out", [2, 128, 2, BS], f16, kind="ExternalOutput").ap()

    with tile.TileContext(nc) as tc:
        with (
            tc.tile_pool(name="res", bufs=1) as res,
            tc.tile_pool(name="work", bufs=1) as work,
            tc.tile_pool(name="pp", bufs=1, space="PSUM") as pp,
        ):
            # PE warm-up identity, built on-chip first (gpsimd owns
            # affine_select; its DMA issues follow right after)
            from concourse.masks import make_identity
            id_sb = res.tile([128, 128], f16, name="id_sb")
            make_identity(nc, id_sb)

            # ---- input DMAs: sync HWDGE ring carries sub 0, gpsimd SWDGE
            # ring carries sub 1.  The scalar ring is kept input-free: its
            # program starts with a ~1.3us ACT_TABLE_LOAD (for the copy
            # activations) which would delay any input bytes behind it. ----
            w_t = []
            for kh in range(2):
                w = res.tile([128, 4, 2, 128], f16, name=f"w{kh}", tag=f"w{kh}")
                (nc.sync if kh == 0 else nc.gpsimd).dma_start(w, wd[kh])
                w_t.append(w)
            e_t = []
            for g in range(8):
                e = res.tile([128, 2, BS], f16, name=f"e{g}", tag=f"e{g}")
                (nc.sync if g < 4 else nc.gpsimd).dma_start(e, encd[g])
                e_t.append(e)

            # keep the HAM activity window busy so the real series starts
            # at the warm clock
            psum_warm = pp.tile([128, 128], f32, name="psum_warm")
            for _ in range(28):
                nc.tensor.matmul(psum_warm, lhsT=id_sb, rhs=id_sb,
                                 start=True, stop=True)

            ps = [[pp.tile([128, BS], f32, name=f"ps{s}{t}") for t in range(2)]
                  for s in range(2)]
            ew_sb = [work.tile([128, 2, BS], f16, name=f"ew{s}")
                     for s in range(2)]

            # k-major, subs interleaved: the series chases piece arrivals on
            # both rings, so after the last piece's semaphore only the final
            # k-pair's matmuls remain
            for k in range(8):
                kq, kl = divmod(k, 2)
                for s in range(2):
                    for t in range(2):
                        nc.tensor.matmul(
                            ps[s][t], lhsT=w_t[k // 4][:, k % 4, t, :],
                            rhs=e_t[s * 4 + kq][:, kl, :],
                            start=(k == 0), stop=(k == 7))
            # psum -> fp16 SBUF on two engines, then ship each sub from its
            # own HWDGE ring so the two out-DMAs overlap
            for s in range(2):
                nc.scalar.activation(ew_sb[s][:, 0, :], ps[s][0], AF.Copy)
                nc.vector.tensor_copy(out=ew_sb[s][:, 1, :], in_=ps[s][1])
                (nc.sync if s == 0 else nc.scalar).dma_start(outd[s], ew_sb[s])

    nc.compile()
    return nc


def _get_nc():
    nc = _NC_CACHE.get("nc")
    if nc is None:
        nc = _build_nc()
        _NC_CACHE["nc"] = nc
    return nc


def _pack_enc(enc_half):
    # [512 b, 1024 k] f32 -> [8, 128, 2, 256] f16, g = sub*4 + kq
    e = np.ascontiguousarray(enc_half.T).astype(np.float16)   # [k, b]
    e = e.reshape(4, 2, 128, 2, BS)           # [kq, kl, p, s, col]
    e = e.transpose(3, 0, 2, 1, 4)            # [s, kq, p, kl, col]
    return np.ascontiguousarray(e.reshape(8, 128, 2, BS))


def _pack_w(Wq):
    # [256 j, 1024 k] f32 -> [2, 128, 4, 2, 128] f16
    w = np.ascontiguousarray(Wq.T).astype(np.float16)         # [k, j]
    w = w.reshape(2, 4, 128, 2, 128)          # [kh, kl, p, jt, j]
    return np.ascontiguousarray(w.transpose(0, 2, 1, 3, 4))


def _host_fallback(enc, h, keys, Uw, Vw, Ww, prelu_a):
    # exact reference math (only used if prelu_a has non-positive entries,
    # where the threshold fold is invalid; never hit for the spec'd inputs)
    gate = 1.0 / (1.0 + np.exp(-(enc @ (h + keys).T)))
    pre = (h @ Uw.T + keys @ Vw.T)[:, None, :] + (enc @ Ww.T)[None, :, :]
    cand = np.where(pre >= 0, pre, prelu_a * pre)
    new = h[:, None, :] + gate.T[:, None, :] * cand
    new = np.where(new == 0, 0.1, new)
    return np.where(new >= 0, np.float32(1.0), np.float32(-1.0)).reshape(
        NB * B, H).astype(np.float32)


def kernel(features, states, Uw, Vw, Ww, keys, prelu_a):
    from concourse import bass_utils

    features = np.asarray(features)
    states = np.asarray(states, dtype=np.float32)
    Uw = np.asarray(Uw, dtype=np.float32)
    Vw = np.asarray(Vw, dtype=np.float32)
    Ww = np.asarray(Ww, dtype=np.float32)
    keys = np.asarray(keys, dtype=np.float32)
    prelu_a = np.asarray(prelu_a, dtype=np.float32)

    enc = np.ascontiguousarray(features[:, 0, :], dtype=np.float32)  # [B, H]
    h = states.reshape(NB, H)

    if np.any(prelu_a <= 0):
        return _host_fallback(enc.astype(np.float64), h.astype(np.float64),
                              keys.astype(np.float64), Uw.astype(np.float64),
                              Vw.astype(np.float64), Ww.astype(np.float64),
                              prelu_a.astype(np.float64))

    # ---- tiny tensors -> per-(i, j) thresholds, in float64 ----
    enc64 = enc.astype(np.float64)
    h64 = h.astype(np.float64)
    k64 = keys.astype(np.float64)
    gateT = 1.0 / (1.0 + np.exp(-(enc64 @ (h64 + k64).T))).T      # [i, j]
    huv = h64 @ Uw.astype(np.float64).T + k64 @ Vw.astype(np.float64).T
    q = -h64 / gateT
    a = prelu_a.astype(np.float64)[None, :]
    t_cand = np.where(q >= 0, q, q / a)
    THR = np.clip(t_cand - huv, -1e30, 1e30).astype(np.float32)   # [i, j]

    nc = _get_nc()

    enc_packs = [_pack_enc(enc[bh * 512:(bh + 1) * 512]) for bh in range(NBH)]
    w_packs = [_pack_w(Ww[jq * 256:(jq + 1) * 256]) for jq in range(NJ)]
    in_maps = []
    for c in range(NCORES):
        jq, bh = divmod(c, 2)
        in_maps.append({
            "encd": enc_packs[bh],
            "wd": w_packs[jq],
        })

    trace = bool(int(os.environ.get("KERNEL_TRACE", "0")))
    res = bass_utils.run_bass_kernel_spmd(
        nc, in_maps, core_ids=list(range(NCORES)), trace=trace)
    kernel.last_result = res

    # gather ew [b, j] from the cores, then apply the thresholds
    ew = np.empty((B, H), dtype=np.float32)
    ev = ew.reshape(NBH, 2, BS, NJ, 2, 128)    # [bh, s, col, jq, t, p]
    for c in range(NCORES):
        jq, bh = divmod(c, 2)
        o = res.results[c]["out"]              # [s, p, t, col] f16
        ev[bh, :, :, jq, :, :] = o.transpose(0, 3, 2, 1)  # [s, col, t, p]
    one = np.float32(1.0)
    neg = np.float32(-1.0)
    full = np.where(ew[None, :, :] >= THR[:, None, :], one, neg)
    return np.ascontiguousarray(full.reshape(NB * B, H), dtype=np.float32)


# revision 17
# speedup vs baseline: 1.0860x; 1.0374x over previous
"""Trainium2 Bass kernel for nn_MemoryCell (scatter_memory).

Full-input contract: kernel(**inputs) takes the complete (unsharded) numpy
inputs and returns the full [NB*B, H] output.

Math (B == H == 1024, NB == 5, T == 128):
    enc  = features[:, 0, :]                         # [B, H] - only slice used
    h    = states.reshape(NB, H)
    gate = sigmoid(enc @ (h + keys).T)               # [B, NB]
    pre  = (h @ Uw.T + keys @ Vw.T)[:, None, :] + (enc @ Ww.T)[None, :, :]
    cand = where(pre >= 0, pre, prelu_a * pre)
    new[i, b, j] = h[i, j] + gate[j, i] * cand[i, b, j]   # B==H broadcast quirk
    out  = sign(new) with exact zeros -> +1, reshaped [NB*B, H]

Because the output is pure signs, the elementwise tail collapses to a
per-(i, j) THRESHOLD on ew = enc @ Ww.T:
    out[i, b, j] = +1  iff  ew[b, j] >= THR[i, j]
with THR = t_cand - huv, t_cand = (q >= 0 ? q : q / a_j), q = -h/gate
(valid for prelu_a > 0; PReLU is monotone there).  gate/huv/THR involve only
O(H*NB) work on tiny tensors and sit on the host (float64, exact), applied
during the gather/unshard step along with the sign expansion.

The device work per core is ONE [512, 1024] x [1024, 256] GEMM in plain fp16
(both operands round-to-nearest fp16: the PE upconverts fp16 to FP22
losslessly, so HW matches the host simulation; together with the fp16
round-trip of ew itself this measures 132 sign flips of 5.24M, rel err
0.010 vs the 0.02 gate) and ships ew back as fp16 (0.26 MB/core).

Sharding: 2 b-halves x 4 j-quarters = 8 cores.  Per core DMA: Ww quarter
(0.5 MB fp16) + enc half (1 MB fp16) in, ew (0.26 MB fp16) out.  Inputs
stream k-chunk-paced on BOTH HWDGE rings (sync + scalar) so the matmul
series chases the arrivals; a short identity warm-up keeps the PE HAM
activity window busy so the series runs at the warm clock.
"""

import os
import numpy as np

H = 1024
B = 1024
NB = 5
NCORES = 8
NJ = 4              # j-quarters of 256 columns
NBH = 2             # b-halves of 512 rows
BS = 256            # b sub-chunk (PSUM tile width)

_NC_CACHE = {}


def _build_nc():
    from concourse import bacc, mybir
    import concourse.tile as tile

    f32 = mybir.dt.float32
    f16 = mybir.dt.float16
    AF = mybir.ActivationFunctionType

    nc = bacc.Bacc("TRN2", debug=False, num_devices=NCORES)

    # g = sub*4 + kq; [p, kl, col] with k = (kq*2 + kl)*128 + p
    encd = nc.dram_tensor("encd", [8, 128, 2, BS], f16, kind="ExternalInput").ap()
    # [khalf, p, kl, jt, j]
    wd = nc.dram_tensor("wd", [2, 128, 4, 2, 128], f16, kind="ExternalInput").ap()
    outd = nc.dram_tensor("out", [2, 128, 2, BS], f16, kind="ExternalOutput").ap()

    with tile.TileContext(nc) as tc:
        with (
            tc.tile_pool(name="res", bufs=1) as res,
            tc.tile_pool(name="work", bufs=1) as work,
            tc.tile_pool(name="pp", bufs=1, space="PSUM") as pp,
        ):
            # PE warm-up identity, built on-chip first (gpsimd owns
            # affine_select; its DMA issues follow right after)
            from concourse.masks import make_identity
            id_sb = res.tile([128, 128], f16, name="id_sb")
            make_identity(nc, id_sb)

            # ---- input DMAs on both HWDGE rings, weights first ----
            w_t = []
            for kh in range(2):
                w = res.tile([128, 4, 2, 128], f16, name=f"w{kh}", tag=f"w{kh}")
                (nc.sync if kh == 0 else nc.scalar).dma_start(w, wd[kh])
                w_t.append(w)
            e_t = []
            for g in range(8):
                e = res.tile([128, 2, BS], f16, name=f"e{g}", tag=f"e{g}")
                # sync ring: s0 pieces; scalar ring: s1 pieces
                (nc.sync if g < 4 else nc.scalar).dma_start(e, encd[g])
                e_t.append(e)

            # keep the HAM activity window busy so the real series starts
            # at the warm clock
            psum_warm = pp.tile([128, 128], f32, name="psum_warm")
            for _ in range(28):
                nc.tensor.matmul(psum_warm, lhsT=id_sb, rhs=id_sb,
                                 start=True, stop=True)

            ps = [[pp.tile([128, BS], f32, name=f"ps{s}{t}") for t in range(2)]
                  for s in range(2)]
            ew_sb = [work.tile([128, 2, BS], f16, name=f"ew{s}")
                     for s in range(2)]

            # k-major, subs interleaved: the series chases piece arrivals on
            # both rings, so after the last piece's semaphore only the final
            # k-pair's matmuls remain
            for k in range(8):
                kq, kl = divmod(k, 2)
                for s in range(2):
                    for t in range(2):
                        nc.tensor.matmul(
                            ps[s][t], lhsT=w_t[k // 4][:, k % 4, t, :],
                            rhs=e_t[s * 4 + kq][:, kl, :],
                            start=(k == 0), stop=(k == 7))
            # psum -> fp16 SBUF on two engines, then ship each sub from its
            # own HWDGE ring so the two out-DMAs overlap
            for s in range(2):
                nc.scalar.activation(ew_sb[s][:, 0, :], ps[s][0], AF.Copy)
                nc.vector.tensor_copy(out=ew_sb[s][:, 1, :], in_=ps[s][1])
                (nc.sync if s == 0 else nc.scalar).dma_start(outd[s], ew_sb[s])

    nc.compile()
    return nc


def _get_nc():
    nc = _NC_CACHE.get("nc")
    if nc is None:
        nc = _build_nc()
        _NC_CACHE["nc"] = nc
    return nc


def _pack_enc(enc_half):
    # [512 b, 1024 k] f32 -> [8, 128, 2, 256] f16, g = sub*4 + kq
    e = np.ascontiguousarray(enc_half.T).astype(np.float16)   # [k, b]
    e = e.reshape(4, 2, 128, 2, BS)           # [kq, kl, p, s, col]
    e = e.transpose(3, 0, 2, 1, 4)            # [s, kq, p, kl, col]
    return np.ascontiguousarray(e.reshape(8, 128, 2, BS))


def _pack_w(Wq):
    # [256 j, 1024 k] f32 -> [2, 128, 4, 2, 128] f16
    w = np.ascontiguousarray(Wq.T).astype(np.float16)         # [k, j]
    w = w.reshape(2, 4, 128, 2, 128)          # [kh, kl, p, jt, j]
    return np.ascontiguousarray(w.transpose(0, 2, 1, 3, 4))


def _host_fallback(enc, h, keys, Uw, Vw, Ww, prelu_a):
    # exact reference math (only used if prelu_a has non-positive entries,
    # where the threshold fold is invalid; never hit for the spec'd inputs)
    gate = 1.0 / (1.0 + np.exp(-(enc @ (h + keys).T)))
    pre = (h @ Uw.T + keys @ Vw.T)[:, None, :] + (enc @ Ww.T)[None, :, :]
    cand = np.where(pre >= 0, pre, prelu_a * pre)
    new = h[:, None, :] + gate.T[:, None, :] * cand
    new = np.where(new == 0, 0.1, new)
    return np.where(new >= 0, np.float32(1.0), np.float32(-1.0)).reshape(
        NB * B, H).astype(np.float32)


def kernel(features, states, Uw, Vw, Ww, keys, prelu_a):
    from concourse import bass_utils

    features = np.asarray(features)
    states = np.asarray(states, dtype=np.float32)
    Uw = np.asarray(Uw, dtype=np.float32)
    Vw = np.asarray(Vw, dtype=np.float32)
    Ww = np.asarray(Ww, dtype=np.float32)
    keys = np.asarray(keys, dtype=np.float32)
    prelu_a = np.asarray(prelu_a, dtype=np.float32)

    enc = np.ascontiguousarray(features[:, 0, :], dtype=np.float32)  # [B, H]
    h = states.reshape(NB, H)

    if np.any(prelu_a <= 0):
        return _host_fallback(enc.astype(np.float64), h.astype(np.float64),
                              keys.astype(np.float64), Uw.astype(np.float64),
                              Vw.astype(np.float64), Ww.astype(np.float64),
                              prelu_a.astype(np.float64))

    # ---- tiny tensors -> per-(i, j) thresholds, in float64 ----
    enc64 = enc.astype(np.float64)
    h64 = h.astype(np.float64)
    k64 = keys.astype(np.float64)
    gateT = 1.0 / (1.0 + np.exp(-(enc64 @ (h64 + k64).T))).T      # [i, j]
    huv = h64 @ Uw.astype(np.float64).T + k64 @ Vw.astype(np.float64).T
    q = -h64 / gateT
    a = prelu_a.astype(np.float64)[None, :]
    t_cand = np.where(q >= 0, q, q / a)
    THR = np.clip(t_cand - huv, -1e30, 1e30).astype(np.float32)   # [i, j]

    nc = _get_nc()

    enc_packs = [_pack_enc(enc[bh * 512:(bh + 1) * 512]) for bh in range(NBH)]
    w_packs = [_pack_w(Ww[jq * 256:(jq + 1) * 256]) for jq in range(NJ)]
    in_maps = []
    for c in range(NCORES):
        jq, bh = divmod(c, 2)
        in_maps.append({
            "encd": enc_packs[bh],
            "wd": w_packs[jq],
        })

    trace = bool(int(os.environ.get("KERNEL_TRACE", "0")))
    res = bass_utils.run_bass_kernel_spmd(
        nc, in_maps, core_ids=list(range(NCORES)), trace=trace)
    kernel.last_result = res

    # gather ew [b, j] from the cores, then apply the thresholds
    ew = np.empty((B, H), dtype=np.float32)
    ev = ew.reshape(NBH, 2, BS, NJ, 2, 128)    # [bh, s, col, jq, t, p]
    for c in range(NCORES):
        jq, bh = divmod(c, 2)
        o = res.results[c]["out"]              # [s, p, t, col] f16
        ev[bh, :, :, jq, :, :] = o.transpose(0, 3, 2, 1)  # [s, col, t, p]
    one = np.float32(1.0)
    neg = np.float32(-1.0)
    full = np.where(ew[None, :, :] >= THR[:, None, :], one, neg)
    return np.ascontiguousarray(full.reshape(NB * B, H), dtype=np.float32)
